# revision 1
# baseline (speedup 1.0000x reference)
"""Trainium2 Bass kernel for nn_DotProcessorBlock.

Computes, for x:[B,N] f32 (B=4096, N=256), w,b:[N]:
    feat = x * w + b                      (elementwise affine on features)
    Z[b,i,j] = feat[b,i] * feat[b,j]      (batched outer product)
    out = Z.reshape(B, N*N)[:, :N*(N+1)//2]   -> [4096, 32896]

Sharding: data-parallel batch split across 8 NeuronCores (512 rows each);
w/b replicated. Host reconstructs the full output from a compact
deduplicated bf16 device output (norm rel err ~3e-3 vs the 2e-2 gate).

Traffic: Z[b] is symmetric, so of the 32896 kept entries per row the
strict lower triangle of the leading 128x128 block and the 128 tail
columns duplicate entries already present. The device writes ~24.8k
elems/row in bf16 (25.4 MB/core vs 67.4 MB full f32).

Compact per-row layout (shared by device and host gather):
- rows 0..29 (ACT): suffix Z[i, j0(i):256], j0(i)=i-(i%2); one
  activation-mul per row (f32 in, bf16 out, per-partition scale);
  ~0.86 ns/elem + ~290 ns/op.
- rows 30..125 (DVE): 24 groups of 4 rows. Group at i0 stores the
  window j in [i0, 256) for its 4 rows INTERLEAVED j-major:
  elem (j, r) at goff + (j-i0)*4 + r = Z[i0+r, j]. One tensor_tensor
  per group: out[p,j,r] = frep4[p,4j+r] * fb16[p,i0+r] where
  frep4[p,4k+r]=feat[p,k] -- every operand is bf16 with innermost
  step 1, so the DVE 2x_1p packed mode engages (HW-measured 0.509
  ns/elem + 166 ns/op, i.e. ~0.52L+42 per row -- beats tensor_scalar's
  0.26L+130 for all L<350).
- rows 126..127 (DVE): per-row suffixes via tensor_scalar (4x mode).

All slice offsets/lengths stay even (4B-aligned) -- required for the
DVE packed modes. Odd ACT rows start one element early (a real
duplicate product); group windows over-cover j in [i0, j0(i)) for rows
i0+2..i0+3 (also real duplicates). Host gather indexes any of them.

Engines balance at ~15 us/tile each (ACT ~14.6, DVE ~15.3 incl feat +
frep4 materialization), matching the output DMA at ~15 us/tile
(16 SDMA engines x ~26 GB/s; chunks issued in completion order so the
ring never head-of-line blocks). The x0|w|b input rides the ACT queue
(first usable after the ~7us framework preamble); later x tiles load
via the gpsimd queue. A dummy 2-elem activation right after the input
load prepays ACT's ~1.3us table load off the critical path.
"""

from contextlib import ExitStack

import numpy as np

import concourse.bacc as bacc
import concourse.tile as tile
from concourse import mybir
from concourse.bass_utils import run_bass_kernel_spmd
from concourse.tile_rust import add_dep_helper

B_FULL = 4096
N = 256
N_CORES = 8
B_CORE = B_FULL // N_CORES          # 512
NUM_INTS = N * (N + 1) // 2         # 32896
P = 128                             # SBUF partitions = batch rows per tile
N_BT = B_CORE // P                  # 4 batch tiles per core

FP32 = mybir.dt.float32
BF16 = mybir.dt.bfloat16

A_ACT = 30                          # rows 0..29 on ACT
G0 = A_ACT                          # first grouped row
NG = 24                             # groups of 4: rows 30..125
T0 = G0 + 4 * NG                    # 126: first trailing TS row

# ---- compact layout tables ----
_J0 = [i - (i % 2) for i in range(P)]          # ACT/TS row start col
_ROW_OFF = np.zeros(P, np.int64)               # per-row base (ACT/TS rows)
_GRP = []                                      # (i0, goff) per group
_off = 0
for _i in range(A_ACT):
    _ROW_OFF[_i] = _off
    _off += N - _J0[_i]
for _g in range(NG):
    _i0 = G0 + 4 * _g
    _GRP.append((_i0, _off))
    _off += 4 * (N - _i0)
for _i in range(T0, P):
    _ROW_OFF[_i] = _off
    _off += N - _J0[_i]
C_TOT = int(_off)                              # 24800


def _src_of(i, j):
    """Compact column holding Z[i, j]; requires j >= cover_start(i)."""
    if i < A_ACT or i >= T0:
        return int(_ROW_OFF[i]) + (j - _J0[i])
    g = (i - G0) // 4
    i0, goff = _GRP[g]
    return goff + (j - i0) * 4 + (i - i0)


def _cover(i):
    if i < A_ACT or i >= T0:
        return _J0[i]
    return _GRP[(i - G0) // 4][0]


def _build_src_index():
    src = np.empty(NUM_INTS, np.int64)
    for i in range(P):
        c = _cover(i)
        for j in range(c, N):
            src[i * N + j] = _src_of(i, j)
        for j in range(c):
            src[i * N + j] = _src_of(j, i)
    for j in range(P):
        src[P * N + j] = _src_of(j, P)
    return src


_SRC = _build_src_index()

# ---- chunk plans ----
# Entries: ("A", row_start, row_end) | ("D", grp_start, grp_end) |
# ("T",) for the two trailing TS rows. Listed in DMA-issue order, which
# approximates completion order on each engine's continuous stream.
_CHUNKS0 = [
    ("D", 0, 2), ("D", 2, 5), ("A", 0, 6), ("D", 5, 11), ("A", 6, 16),
    ("D", 11, 18), ("A", 16, 30), ("D", 18, 24), ("T",),
]
_CHUNKSM = [
    ("D", 0, 5), ("D", 5, 11), ("A", 0, 15), ("D", 11, 18),
    ("A", 15, 30), ("D", 18, 24), ("T",),
]


def _plan_cols(ch):
    """(c0, csz) of a chunk in the compact layout."""
    kind = ch[0]
    if kind == "A":
        c0 = int(_ROW_OFF[ch[1]])
        end = int(_ROW_OFF[ch[2]]) if ch[2] < A_ACT else int(_GRP[0][1])
        return c0, end - c0
    if kind == "D":
        c0 = int(_GRP[ch[1]][1])
        end = int(_GRP[ch[2]][1]) if ch[2] < NG else int(_ROW_OFF[T0])
        return c0, end - c0
    return int(_ROW_OFF[T0]), C_TOT - int(_ROW_OFF[T0])


def _check_plan(plan):
    cols = sorted(_plan_cols(ch) for ch in plan)
    pos = 0
    for c0, csz in cols:
        assert c0 == pos, (c0, pos)
        pos += csz
    assert pos == C_TOT, pos


_check_plan(_CHUNKS0)
_check_plan(_CHUNKSM)


def _emit(ctx, tc, cout, x0wb, xr):
    nc = tc.nc
    const_pool = ctx.enter_context(tc.tile_pool(name="const", bufs=1))
    x_pool = ctx.enter_context(tc.tile_pool(name="x", bufs=4))
    f_pool = ctx.enter_context(tc.tile_pool(name="feat", bufs=2))
    fb_pool = ctx.enter_context(tc.tile_pool(name="featb", bufs=2))
    fr_pool = ctx.enter_context(tc.tile_pool(name="frep", bufs=2))
    o_pool = ctx.enter_context(tc.tile_pool(name="out", bufs=10))

    x0wb_t = const_pool.tile([P, 3 * N], FP32, tag="x0wb")
    nc.scalar.dma_start(x0wb_t[:], x0wb[:])
    x0_t = x0wb_t[:, 0:N]
    w_t = x0wb_t[:, N:2 * N]
    b_t = x0wb_t[:, 2 * N:3 * N]
    # Prepay ACT's ~1.3us activation-table load while feat is computed.
    warm = const_pool.tile([P, 2], FP32, tag="warm")
    nc.scalar.mul(warm[:], x0wb_t[:, 0:2], x0wb_t[:, 0:1])

    def load_feat(bt, order_after=None):
        feat = f_pool.tile([P, N], FP32, tag="feat")
        fb16 = fb_pool.tile([P, N], BF16, tag="fb16")
        frep4 = fr_pool.tile([P, 4 * N], BF16, tag="frep4")
        if bt == 0:
            x_t = x0_t
        else:
            x_tile = x_pool.tile([P, N], FP32, tag="x")
            nc.gpsimd.dma_start(x_tile[:], xr[(bt - 1) * P:bt * P, :])
            x_t = x_tile[:]
        mul = nc.vector.tensor_mul(feat[:], x_t, w_t)
        if order_after is not None:
            # Order-only edge: keep the next feat's DVE ops from being
            # statically scheduled ahead of the chunk-critical group ops.
            add_dep_helper(mul.ins, order_after.ins, sync=False,
                           reason="chunk groups first on DVE")
        nc.vector.tensor_add(feat[:], feat[:], b_t)
        nc.vector.tensor_copy(fb16[:], feat[:])
        # frep4[p, 4k+r] = feat[p, k] (bf16) -- group-op operand.
        nc.vector.tensor_copy(
            frep4[:].rearrange("p (k r) -> p k r", k=N, r=4),
            fb16[:].unsqueeze(2).broadcast_to((P, N, 4)))
        return feat, fb16, frep4

    feat, fb16, frep4 = load_feat(0)
    for bt in range(N_BT):
        plan = _CHUNKS0 if bt == 0 else _CHUNKSM
        next_ld = None
        n_dve_chunks = 0
        for ch in plan:
            c0, csz = _plan_cols(ch)
            ot = o_pool.tile([P, csz], BF16, tag="ot")
            last_op = None
            if ch[0] == "A":
                for i in range(ch[1], ch[2]):
                    o0 = int(_ROW_OFF[i]) - c0
                    L = N - _J0[i]
                    nc.scalar.mul(ot[:, o0:o0 + L],
                                  feat[:, _J0[i]:N], feat[:, i:i + 1])
            elif ch[0] == "D":
                for g in range(ch[1], ch[2]):
                    i0, goff = _GRP[g]
                    Lg = N - i0
                    o0 = goff - c0
                    out3 = ot[:, o0:o0 + 4 * Lg].rearrange(
                        "p (j r) -> p j r", j=Lg, r=4)
                    in0 = frep4[:, 4 * i0:4 * N].rearrange(
                        "p (j r) -> p j r", j=Lg, r=4)
                    in1 = fb16[:, i0:i0 + 4].unsqueeze(1).broadcast_to(
                        (P, Lg, 4))
                    last_op = nc.vector.tensor_mul(out3, in0, in1)
            else:  # trailing TS rows 126..127
                for i in range(T0, P):
                    o0 = int(_ROW_OFF[i]) - c0
                    L = N - _J0[i]
                    last_op = nc.vector.tensor_scalar_mul(
                        ot[:, o0:o0 + L], fb16[:, _J0[i]:N],
                        feat[:, i:i + 1])
            nc.sync.dma_start(cout[bt * P:(bt + 1) * P, c0:c0 + csz],
                              ot[:, :csz])
            if ch[0] == "D":
                n_dve_chunks += 1
                if n_dve_chunks == 3 and bt + 1 < N_BT:
                    next_ld = load_feat(bt + 1, order_after=last_op)
        if next_ld is not None:
            feat, fb16, frep4 = next_ld


def _build():
    nc = bacc.Bacc("TRN2", target_bir_lowering=False, debug=False,
                   num_devices=N_CORES)
    x0wb = nc.dram_tensor("x0wb", [P, 3 * N], FP32, kind="ExternalInput").ap()
    xr = nc.dram_tensor("xr", [B_CORE - P, N], FP32,
                        kind="ExternalInput").ap()
    cout = nc.dram_tensor("cout", [B_CORE, C_TOT], BF16,
                          kind="ExternalOutput").ap()
    with tile.TileContext(nc) as tc, ExitStack() as ctx:
        _emit(ctx, tc, cout, x0wb, xr)
    nc.compile()
    return nc


_NC_CACHE = None


def _get_nc():
    global _NC_CACHE
    if _NC_CACHE is None:
        _NC_CACHE = _build()
    return _NC_CACHE


def run(x, weight_w, weight_b, trace=False, **run_kwargs):
    x = np.ascontiguousarray(np.asarray(x, dtype=np.float32))
    w = np.asarray(weight_w, dtype=np.float32).reshape(N)
    b = np.asarray(weight_b, dtype=np.float32).reshape(N)
    assert x.shape == (B_FULL, N), x.shape

    wb = np.broadcast_to(np.concatenate([w, b]), (P, 2 * N))
    in_maps = []
    for i in range(N_CORES):
        xs = x[i * B_CORE:(i + 1) * B_CORE]
        in_maps.append({
            "x0wb": np.ascontiguousarray(np.hstack([xs[:P], wb])),
            "xr": xs[P:],
        })
    res = run_bass_kernel_spmd(
        _get_nc(), in_maps, core_ids=list(range(N_CORES)), trace=trace,
        **run_kwargs,
    )
    compact = np.concatenate([r["cout"] for r in res.results], axis=0)
    assert compact.shape == (B_FULL, C_TOT), compact.shape
    full = compact[:, _SRC].astype(np.float32)
    return full, res


def kernel(x, weight_w, weight_b):
    full, _ = run(x, weight_w, weight_b, trace=False)
    return full

